# revision 20
# baseline (speedup 1.0000x reference)
"""MoE ResNet BK layer — Trainium2 Bass kernel (8 NeuronCores, expert-parallel).

Strategy:
  * Expert-parallel: core e holds expert e's (w1, b1, w2, b2) resident in SBUF.
  * Host computes the top-2 softmax gating and routes tokens: core e receives
    only the tokens whose top-2 set contains expert e (padded to capacity C),
    pre-transposed to [D, C] so the device keeps the hidden activations in
    [feature, token] layout throughout.
  * Device per core: h = gelu_tanh(w1e.T @ xg + b1e) in [F, tok] layout,
    y = w2e.T @ h + b2e in [D, tok] layout. Matmuls run as float32r
    (full-rate fp32, ~FP22 multiply precision, fp32 accumulate in PSUM).
  * Host combines: out[tok] = sum over the token's 2 experts of cw * y,
    plus the (cheap, serial) BK tridiagonal Green's-function branch.
"""

import numpy as np

B, N, D, E, F, K = 2, 2048, 768, 8, 3072, 2
V_MAX = 3.0
FEAT_CLAMP = 10.0
NCORES = 8
KD = D // 128   # 6
KF = F // 128   # 24

LAST_RESULTS = None  # BassKernelResults of the most recent device run


# ---------------------------------------------------------------- host pieces

def _softmax(x, axis=-1):
    m = np.max(x, axis=axis, keepdims=True)
    p = np.exp(x - m)
    return p / np.sum(p, axis=axis, keepdims=True)


def _route(x_flat, gate_w, gate_b):
    """Top-2 routing. Returns (top1, top2, cw1, cw2) per token."""
    logits = x_flat.astype(np.float32) @ gate_w.astype(np.float32) + gate_b
    probs = _softmax(logits, axis=-1)
    t = np.arange(probs.shape[0])
    top1 = np.argmax(probs, axis=-1)
    p2 = probs.copy()
    p2[t, top1] = -np.inf
    top2 = np.argmax(p2, axis=-1)
    v1 = probs[t, top1]
    v2 = probs[t, top2]
    s = v1 + v2
    return top1, top2, (v1 / s).astype(np.float32), (v2 / s).astype(np.float32)


def _bk_feats(x_flat, v_w, v_b):
    """BK tridiagonal Green's function diagonal -> (B*N, 2) float32 feats."""
    v = np.clip(x_flat @ v_w.astype(np.float64) + np.float64(v_b), -V_MAX, V_MAX)
    a = (-2.0 + v).astype(np.complex128) - 1j           # (B*N,)
    aT = a.reshape(B, N).T                              # (N, B)
    # Forward scan (d) and reverse scan (e) share one python loop.
    s = np.concatenate([aT, aT[::-1]], axis=1)          # (N, 2B)
    r = np.empty_like(s)
    r[0] = s[0]
    for i in range(1, N):
        r[i] = s[i] - 1.0 / r[i - 1]
    d = r[:, :B]
    e = r[::-1, B:]
    G = (1.0 / (d + e - aT)).T                          # (B, N)
    feats = np.stack([G.real, G.imag], axis=-1).astype(np.float32)
    return feats.reshape(B * N, 2)


def _round_fp22(a):
    """Round float32 -> fp32r (FP22: 13 mantissa bits), round-to-nearest-even."""
    b = np.ascontiguousarray(a, np.float32).view(np.uint32)
    keep = np.uint32(10)  # drop 10 low mantissa bits
    half = np.uint32(1 << 9)
    lsb = (b >> keep) & np.uint32(1)
    r = (b + (half - np.uint32(1)) + lsb) & np.uint32(~((1 << 10) - 1) & 0xFFFFFFFF)
    return r.view(np.float32)


# ---------------------------------------------------------------- device part

_BASS_CACHE = {}


def _build_bass(C, tiles):
    """Per-core expert-FFN program: yt = (gelu(xg@w1+b1) @ w2 + b2) in [D, C]."""
    import concourse.bacc as bacc
    import concourse.tile as tile
    import concourse.mybir as mybir

    f32 = mybir.dt.float32
    f32r = mybir.dt.float32r
    GELU = mybir.ActivationFunctionType.Gelu_apprx_tanh

    nc = bacc.Bacc("TRN2", target_bir_lowering=False, debug=False,
                   num_devices=NCORES)
    xt_d = nc.dram_tensor("xt", [D, C], f32r, kind="ExternalInput").ap()
    # w1 arrives pre-packed: row f*128+p holds w1[k*128+p, f*128+m] for all
    # (k, m), so each per-f tile DMA is one contiguous 384 KB burst.
    w1_d = nc.dram_tensor("w1e", [F, D], f32r, kind="ExternalInput").ap()
    w2_d = nc.dram_tensor("w2e", [F, D], f32r, kind="ExternalInput").ap()
    b1_d = nc.dram_tensor("b1c", [128, KF], f32, kind="ExternalInput").ap()
    b2_d = nc.dram_tensor("b2c", [128, KD], f32, kind="ExternalInput").ap()
    yt_d = nc.dram_tensor("yt", [D, C], f32, kind="ExternalOutput").ap()
    y_v = yt_d.rearrange("(d p) n -> p d n", p=128)          # [128, KD, C]

    # DRAM view tiled to 128-partition blocks.
    x_v = xt_d.rearrange("(k p) n -> p k n", p=128)          # [128, KD, C]

    with tile.TileContext(nc) as tc:
        with (
            tc.tile_pool(name="wts", bufs=1) as wpool,
            tc.tile_pool(name="xin", bufs=1) as xpool,
            tc.tile_pool(name="hbuf", bufs=3) as hpool,
            tc.tile_pool(name="yout", bufs=1) as ypool,
            tc.tile_pool(name="ph", bufs=2, space="PSUM") as phpool,
            tc.tile_pool(name="py", bufs=1, space="PSUM") as pypool,
        ):
            # PE warm-up: dummy fp32 matmuls keep the PE instruction stream
            # hot and flip the HAM clock-gate to 8/8 while the first
            # weight/x DMAs are still in flight.
            dmx = wpool.tile([128, 512], f32, tag="dmx")
            nc.gpsimd.memset(dmx[:], 0.0)
            dps = phpool.tile([128, 128], f32, tag="ph", name="dps")
            for i in range(12):
                nc.tensor.matmul(dps[:], dmx[:, 0:128], dmx[:, 128:256],
                                 start=(i == 0), stop=(i == 11))

            b1_sb = wpool.tile([128, KF], f32, tag="b1")
            b2_sb = wpool.tile([128, KD], f32, tag="b2")
            w1f = [wpool.tile([128, KD, 128], f32r, tag=f"w1_{f}",
                              name=f"w1_{f}") for f in range(KF)]
            w2f = [wpool.tile([128, D], f32r, tag=f"w2_{f}", name=f"w2_{f}")
                   for f in range(KF)]
            xts = [xpool.tile([128, KD, NT], f32r, tag=f"x{i}", name=f"x{i}")
                   for i, (t0, NT) in enumerate(tiles)]

            # All loads go through the single Sync HWDGE queue in explicit
            # priority order (one queue already sustains full HBM read BW;
            # parallel queues only starve the critical weight stream).
            def dma_x(i, k):
                t0, NT = tiles[i]
                if k is None:
                    nc.sync.dma_start(xts[i][:], x_v[:, :, t0:t0 + NT])
                else:
                    nc.sync.dma_start(xts[i][:, k, :],
                                      x_v[:, k, t0:t0 + NT])

            # x-tile i>0 is consumed ~60us per preceding tile; slot its
            # load into the weight stream well before it is needed.
            x_after = {}
            for i in range(1, len(tiles)):
                x_after.setdefault(min(10 * i, KF - 1), []).append(i)
            dma_x(0, 0)
            for f in range(KF):
                nc.sync.dma_start(w1f[f][:],
                                  w1_d[f * 128:(f + 1) * 128, :])
                if f == 0:
                    dma_x(0, 1)
                    nc.sync.dma_start(b1_sb[:], b1_d[:])
                    nc.sync.dma_start(b2_sb[:], b2_d[:])
                    t0_, NT_ = tiles[0]
                    nc.sync.dma_start(xts[0][:, 2:, :],
                                      x_v[:, 2:, t0_:t0_ + NT_])
                nc.sync.dma_start(w2f[f][:], w2_d[f * 128:(f + 1) * 128, :])
                for i in x_after.get(f, ()):
                    dma_x(i, None)

            for i, (t0, NT) in enumerate(tiles):
                xt_sb = xts[i]
                pys = [pypool.tile([128, NT], f32, tag=f"py{d}", name=f"py{d}")
                       for d in range(KD)]
                pending = None  # (h_tile, f) whose MM2 is not yet emitted
                for f in range(KF):
                    ph = phpool.tile([128, NT], f32, tag="ph")
                    for k in range(KD):
                        nc.tensor.matmul(
                            ph[:],
                            w1f[f][:, k, :],
                            xt_sb[:, k, :],
                            start=(k == 0), stop=(k == KD - 1),
                        )
                    hf = hpool.tile([128, NT], f32r, tag="h")
                    nc.scalar.activation(hf[:], ph[:], GELU,
                                         bias=b1_sb[:, f:f + 1])
                    if pending is not None:
                        hp, fp = pending
                        for d in range(KD):
                            nc.tensor.matmul(
                                pys[d][:],
                                w2f[fp][:, d * 128:(d + 1) * 128],
                                hp[:],
                                start=(fp == 0), stop=False,
                            )
                    pending = (hf, f)
                hp, fp = pending
                for d in range(KD):
                    nc.tensor.matmul(
                        pys[d][:],
                        w2f[fp][:, d * 128:(d + 1) * 128],
                        hp[:],
                        start=False, stop=True,
                    )
                IDENT = mybir.ActivationFunctionType.Identity
                y_all = ypool.tile([128, KD, NT], f32, tag="y")
                for d in (0, 1, 2, 3, 4, 5):
                    # Split the PSUM-draining b2-add across DVE and ACT so
                    # the psum banks free up ~2x faster for the next tile.
                    if d % 2:
                        nc.scalar.activation(y_all[:, d, :], pys[d][:], IDENT,
                                             bias=b2_sb[:, d:d + 1])
                    else:
                        nc.vector.tensor_scalar_add(y_all[:, d, :], pys[d][:],
                                                    b2_sb[:, d:d + 1])
                half = KD // 2
                nc.sync.dma_start(y_v[:, :half, t0:t0 + NT],
                                  y_all[:, :half, :])
                nc.sync.dma_start(y_v[:, half:, t0:t0 + NT],
                                  y_all[:, half:, :])
    nc.compile()
    return nc


def _token_tiles(C):
    """Decompose C into moving-dim tiles, all >=256 (fp32r full rate)."""
    tiles = []
    t0 = 0
    rem = C
    while rem >= 768:
        tiles.append((t0, 512))
        t0 += 512
        rem -= 512
    if rem > 512:
        a = rem - 256
        tiles += [(t0, a), (t0 + a, 256)]
    elif rem > 0:
        tiles.append((t0, rem))
    return tiles


# ---------------------------------------------------------------- entry point

def kernel(x, v_w, v_b, gate_w, gate_b, w1, b1, w2, b2, out_w, out_b, bk_scale):
    global LAST_RESULTS
    from concourse.bass_utils import run_bass_kernel_spmd

    x = np.asarray(x, np.float32)
    v_w = np.asarray(v_w, np.float32)
    v_b = np.float32(np.asarray(v_b))
    gate_w = np.asarray(gate_w, np.float32)
    gate_b = np.asarray(gate_b, np.float32)
    w1 = np.asarray(w1, np.float32)
    b1 = np.asarray(b1, np.float32)
    w2 = np.asarray(w2, np.float32)
    b2 = np.asarray(b2, np.float32)
    out_w = np.asarray(out_w, np.float32)
    out_b = np.asarray(out_b, np.float32)
    bk_scale = np.asarray(bk_scale, np.float32)

    T = B * N
    x_flat = np.ascontiguousarray(x.reshape(T, D))

    # --- host: gating / routing ---
    top1, top2, cw1, cw2 = _route(x_flat, gate_w, gate_b)
    idx = []
    cws = []
    for e in range(E):
        m1 = top1 == e
        m2 = top2 == e
        ie = np.nonzero(m1 | m2)[0]
        ce = np.where(m1[ie], cw1[ie], cw2[ie]).astype(np.float32)
        idx.append(ie)
        cws.append(ce)
    max_load = max(len(ie) for ie in idx)
    C = max(256, -(-max_load // 8) * 8)
    tiles = _token_tiles(C)

    # --- device: expert FFN on 8 cores ---
    key = (C,)
    nc = _BASS_CACHE.get(key)
    if nc is None:
        nc = _build_bass(C, tiles)
        _BASS_CACHE[key] = nc

    in_maps = []
    for e in range(E):
        ie = idx[e]
        xt = np.zeros((D, C), np.float32)
        xt[:, :len(ie)] = _round_fp22(x_flat[ie].T)
        in_maps.append({
            "xt": xt,
            "w1e": _round_fp22(w1[e].reshape(KD, 128, KF, 128)
                               .transpose(2, 1, 0, 3).reshape(F, D)),
            "w2e": _round_fp22(w2[e]),
            "b1c": np.ascontiguousarray(b1[e].reshape(KF, 128).T),
            "b2c": np.ascontiguousarray(b2[e].reshape(KD, 128).T),
        })

    res = run_bass_kernel_spmd(nc, in_maps, core_ids=list(range(NCORES)))
    LAST_RESULTS = res

    # --- host: combine expert outputs ---
    out_flat = np.zeros((T, D), np.float32)
    for e in range(E):
        ie = idx[e]
        ye = res.results[e]["yt"][:, :len(ie)].T   # (n_e, D)
        out_flat[ie] += cws[e][:, None] * ye

    # --- host: BK spectral branch ---
    feats = _bk_feats(x_flat.astype(np.float64), v_w, v_b)
    feats_c = np.clip(feats, -FEAT_CLAMP, FEAT_CLAMP)
    spec = feats_c @ out_w + out_b                  # (T, D)
    out_flat += bk_scale[None, :] * spec

    return out_flat.reshape(B, N, D)


# revision 21
# speedup vs baseline: 1.0222x; 1.0222x over previous
"""MoE ResNet BK layer — Trainium2 Bass kernel (8 NeuronCores, expert-parallel).

Strategy:
  * Expert-parallel: core e holds expert e's (w1, b1, w2, b2) resident in SBUF.
  * Host computes the top-2 softmax gating and routes tokens: core e receives
    only the tokens whose top-2 set contains expert e (padded to capacity C),
    pre-transposed to [D, C] so the device keeps the hidden activations in
    [feature, token] layout throughout.
  * Device per core: h = gelu_tanh(w1e.T @ xg + b1e) in [F, tok] layout,
    y = w2e.T @ h + b2e in [D, tok] layout. Matmuls run as float32r
    (full-rate fp32, ~FP22 multiply precision, fp32 accumulate in PSUM).
  * Host combines: out[tok] = sum over the token's 2 experts of cw * y,
    plus the (cheap, serial) BK tridiagonal Green's-function branch.
"""

import numpy as np

B, N, D, E, F, K = 2, 2048, 768, 8, 3072, 2
V_MAX = 3.0
FEAT_CLAMP = 10.0
NCORES = 8
KD = D // 128   # 6
KF = F // 128   # 24

LAST_RESULTS = None  # BassKernelResults of the most recent device run


# ---------------------------------------------------------------- host pieces

def _softmax(x, axis=-1):
    m = np.max(x, axis=axis, keepdims=True)
    p = np.exp(x - m)
    return p / np.sum(p, axis=axis, keepdims=True)


def _route(x_flat, gate_w, gate_b):
    """Top-2 routing. Returns (top1, top2, cw1, cw2) per token."""
    logits = x_flat.astype(np.float32) @ gate_w.astype(np.float32) + gate_b
    probs = _softmax(logits, axis=-1)
    t = np.arange(probs.shape[0])
    top1 = np.argmax(probs, axis=-1)
    p2 = probs.copy()
    p2[t, top1] = -np.inf
    top2 = np.argmax(p2, axis=-1)
    v1 = probs[t, top1]
    v2 = probs[t, top2]
    s = v1 + v2
    return top1, top2, (v1 / s).astype(np.float32), (v2 / s).astype(np.float32)


def _bk_feats(x_flat, v_w, v_b):
    """BK tridiagonal Green's function diagonal -> (B*N, 2) float32 feats."""
    v = np.clip(x_flat @ v_w.astype(np.float64) + np.float64(v_b), -V_MAX, V_MAX)
    a = (-2.0 + v).astype(np.complex128) - 1j           # (B*N,)
    aT = a.reshape(B, N).T                              # (N, B)
    # Forward scan (d) and reverse scan (e) share one python loop.
    s = np.concatenate([aT, aT[::-1]], axis=1)          # (N, 2B)
    r = np.empty_like(s)
    r[0] = s[0]
    for i in range(1, N):
        r[i] = s[i] - 1.0 / r[i - 1]
    d = r[:, :B]
    e = r[::-1, B:]
    G = (1.0 / (d + e - aT)).T                          # (B, N)
    feats = np.stack([G.real, G.imag], axis=-1).astype(np.float32)
    return feats.reshape(B * N, 2)


def _round_fp22(a):
    """Round float32 -> fp32r (FP22: 13 mantissa bits), round-to-nearest-even."""
    b = np.ascontiguousarray(a, np.float32).view(np.uint32)
    keep = np.uint32(10)  # drop 10 low mantissa bits
    half = np.uint32(1 << 9)
    lsb = (b >> keep) & np.uint32(1)
    r = (b + (half - np.uint32(1)) + lsb) & np.uint32(~((1 << 10) - 1) & 0xFFFFFFFF)
    return r.view(np.float32)


# ---------------------------------------------------------------- device part

_BASS_CACHE = {}


def _build_bass(C, tiles):
    """Per-core expert-FFN program: yt = (gelu(xg@w1+b1) @ w2 + b2) in [D, C]."""
    import concourse.bacc as bacc
    import concourse.tile as tile
    import concourse.mybir as mybir

    f32 = mybir.dt.float32
    f32r = mybir.dt.float32r
    GELU = mybir.ActivationFunctionType.Gelu_apprx_tanh

    nc = bacc.Bacc("TRN2", target_bir_lowering=False, debug=False,
                   num_devices=NCORES)
    xt_d = nc.dram_tensor("xt", [D, C], f32r, kind="ExternalInput").ap()
    # w1 arrives pre-packed: row f*128+p holds w1[k*128+p, f*128+m] for all
    # (k, m), so each per-f tile DMA is one contiguous 384 KB burst.
    w1_d = nc.dram_tensor("w1e", [F, D], f32r, kind="ExternalInput").ap()
    w2_d = nc.dram_tensor("w2e", [F, D], f32r, kind="ExternalInput").ap()
    b1_d = nc.dram_tensor("b1c", [128, KF], f32, kind="ExternalInput").ap()
    b2_d = nc.dram_tensor("b2c", [128, KD], f32, kind="ExternalInput").ap()
    yt_d = nc.dram_tensor("yt", [D, C], f32, kind="ExternalOutput").ap()
    y_v = yt_d.rearrange("(d p) n -> p d n", p=128)          # [128, KD, C]

    # DRAM view tiled to 128-partition blocks.
    x_v = xt_d.rearrange("(k p) n -> p k n", p=128)          # [128, KD, C]

    with tile.TileContext(nc) as tc:
        with (
            tc.tile_pool(name="wts", bufs=1) as wpool,
            tc.tile_pool(name="xin", bufs=1) as xpool,
            tc.tile_pool(name="hbuf", bufs=3) as hpool,
            tc.tile_pool(name="yout", bufs=1) as ypool,
            tc.tile_pool(name="ph", bufs=2, space="PSUM") as phpool,
            tc.tile_pool(name="py", bufs=1, space="PSUM") as pypool,
        ):
            # PE warm-up: dummy fp32 matmuls keep the PE instruction stream
            # hot and flip the HAM clock-gate to 8/8 while the first
            # weight/x DMAs are still in flight.
            dmx = wpool.tile([128, 512], f32, tag="dmx")
            nc.gpsimd.memset(dmx[:], 0.0)
            dps = phpool.tile([128, 128], f32, tag="ph", name="dps")
            for i in range(12):
                nc.tensor.matmul(dps[:], dmx[:, 0:128], dmx[:, 128:256],
                                 start=(i == 0), stop=(i == 11))

            b1_sb = wpool.tile([128, KF], f32, tag="b1")
            b2_sb = wpool.tile([128, KD], f32, tag="b2")
            w1f = [wpool.tile([128, KD, 128], f32r, tag=f"w1_{f}",
                              name=f"w1_{f}") for f in range(KF)]
            w2f = [wpool.tile([128, D], f32r, tag=f"w2_{f}", name=f"w2_{f}")
                   for f in range(KF)]
            xts = [xpool.tile([128, KD, NT], f32r, tag=f"x{i}", name=f"x{i}")
                   for i, (t0, NT) in enumerate(tiles)]

            # All loads go through the single Sync HWDGE queue in explicit
            # priority order (one queue already sustains full HBM read BW;
            # parallel queues only starve the critical weight stream).
            def dma_x(i, k):
                t0, NT = tiles[i]
                if k is None:
                    nc.sync.dma_start(xts[i][:], x_v[:, :, t0:t0 + NT])
                else:
                    nc.sync.dma_start(xts[i][:, k, :],
                                      x_v[:, k, t0:t0 + NT])

            # x-tile i>0 is consumed ~60us per preceding tile; slot its
            # load into the weight stream well before it is needed.
            x_after = {}
            for i in range(1, len(tiles)):
                x_after.setdefault(min(10 * i, KF - 1), []).append(i)
            # The compute consumes weights staggered -- MM1(f+1) issues
            # before MM2(f) -- so deliver w1 one step ahead of w2.
            dma_x(0, 0)
            nc.sync.dma_start(w1f[0][:], w1_d[0:128, :])
            dma_x(0, 1)
            nc.sync.dma_start(b1_sb[:], b1_d[:])
            nc.sync.dma_start(b2_sb[:], b2_d[:])
            t0_, NT_ = tiles[0]
            nc.sync.dma_start(xts[0][:, 2:, :], x_v[:, 2:, t0_:t0_ + NT_])
            for f in range(KF):
                if f + 1 < KF:
                    nc.sync.dma_start(w1f[f + 1][:],
                                      w1_d[(f + 1) * 128:(f + 2) * 128, :])
                nc.sync.dma_start(w2f[f][:], w2_d[f * 128:(f + 1) * 128, :])
                for i in x_after.get(f, ()):
                    dma_x(i, None)

            for i, (t0, NT) in enumerate(tiles):
                xt_sb = xts[i]
                pys = [pypool.tile([128, NT], f32, tag=f"py{d}", name=f"py{d}")
                       for d in range(KD)]
                pending = None  # (h_tile, f) whose MM2 is not yet emitted
                for f in range(KF):
                    ph = phpool.tile([128, NT], f32, tag="ph")
                    for k in range(KD):
                        nc.tensor.matmul(
                            ph[:],
                            w1f[f][:, k, :],
                            xt_sb[:, k, :],
                            start=(k == 0), stop=(k == KD - 1),
                        )
                    hf = hpool.tile([128, NT], f32r, tag="h")
                    nc.scalar.activation(hf[:], ph[:], GELU,
                                         bias=b1_sb[:, f:f + 1])
                    if pending is not None:
                        hp, fp = pending
                        for d in range(KD):
                            nc.tensor.matmul(
                                pys[d][:],
                                w2f[fp][:, d * 128:(d + 1) * 128],
                                hp[:],
                                start=(fp == 0), stop=False,
                            )
                    pending = (hf, f)
                hp, fp = pending
                for d in range(KD):
                    nc.tensor.matmul(
                        pys[d][:],
                        w2f[fp][:, d * 128:(d + 1) * 128],
                        hp[:],
                        start=False, stop=True,
                    )
                IDENT = mybir.ActivationFunctionType.Identity
                y_all = ypool.tile([128, KD, NT], f32, tag="y")
                for d in (0, 1, 2, 3, 4, 5):
                    # Split the PSUM-draining b2-add across DVE and ACT so
                    # the psum banks free up ~2x faster for the next tile.
                    if d % 2:
                        nc.scalar.activation(y_all[:, d, :], pys[d][:], IDENT,
                                             bias=b2_sb[:, d:d + 1])
                    else:
                        nc.vector.tensor_scalar_add(y_all[:, d, :], pys[d][:],
                                                    b2_sb[:, d:d + 1])
                half = KD // 2
                nc.sync.dma_start(y_v[:, :half, t0:t0 + NT],
                                  y_all[:, :half, :])
                nc.sync.dma_start(y_v[:, half:, t0:t0 + NT],
                                  y_all[:, half:, :])
    nc.compile()
    return nc


def _token_tiles(C):
    """Decompose C into moving-dim tiles, all >=256 (fp32r full rate)."""
    tiles = []
    t0 = 0
    rem = C
    while rem >= 768:
        tiles.append((t0, 512))
        t0 += 512
        rem -= 512
    if rem > 512:
        a = rem - 256
        tiles += [(t0, a), (t0 + a, 256)]
    elif rem > 0:
        tiles.append((t0, rem))
    return tiles


# ---------------------------------------------------------------- entry point

def kernel(x, v_w, v_b, gate_w, gate_b, w1, b1, w2, b2, out_w, out_b, bk_scale):
    global LAST_RESULTS
    from concourse.bass_utils import run_bass_kernel_spmd

    x = np.asarray(x, np.float32)
    v_w = np.asarray(v_w, np.float32)
    v_b = np.float32(np.asarray(v_b))
    gate_w = np.asarray(gate_w, np.float32)
    gate_b = np.asarray(gate_b, np.float32)
    w1 = np.asarray(w1, np.float32)
    b1 = np.asarray(b1, np.float32)
    w2 = np.asarray(w2, np.float32)
    b2 = np.asarray(b2, np.float32)
    out_w = np.asarray(out_w, np.float32)
    out_b = np.asarray(out_b, np.float32)
    bk_scale = np.asarray(bk_scale, np.float32)

    T = B * N
    x_flat = np.ascontiguousarray(x.reshape(T, D))

    # --- host: gating / routing ---
    top1, top2, cw1, cw2 = _route(x_flat, gate_w, gate_b)
    idx = []
    cws = []
    for e in range(E):
        m1 = top1 == e
        m2 = top2 == e
        ie = np.nonzero(m1 | m2)[0]
        ce = np.where(m1[ie], cw1[ie], cw2[ie]).astype(np.float32)
        idx.append(ie)
        cws.append(ce)
    max_load = max(len(ie) for ie in idx)
    C = max(256, -(-max_load // 8) * 8)
    tiles = _token_tiles(C)

    # --- device: expert FFN on 8 cores ---
    key = (C,)
    nc = _BASS_CACHE.get(key)
    if nc is None:
        nc = _build_bass(C, tiles)
        _BASS_CACHE[key] = nc

    in_maps = []
    for e in range(E):
        ie = idx[e]
        xt = np.zeros((D, C), np.float32)
        xt[:, :len(ie)] = _round_fp22(x_flat[ie].T)
        in_maps.append({
            "xt": xt,
            "w1e": _round_fp22(w1[e].reshape(KD, 128, KF, 128)
                               .transpose(2, 1, 0, 3).reshape(F, D)),
            "w2e": _round_fp22(w2[e]),
            "b1c": np.ascontiguousarray(b1[e].reshape(KF, 128).T),
            "b2c": np.ascontiguousarray(b2[e].reshape(KD, 128).T),
        })

    res = run_bass_kernel_spmd(nc, in_maps, core_ids=list(range(NCORES)))
    LAST_RESULTS = res

    # --- host: combine expert outputs ---
    out_flat = np.zeros((T, D), np.float32)
    for e in range(E):
        ie = idx[e]
        ye = res.results[e]["yt"][:, :len(ie)].T   # (n_e, D)
        out_flat[ie] += cws[e][:, None] * ye

    # --- host: BK spectral branch ---
    feats = _bk_feats(x_flat.astype(np.float64), v_w, v_b)
    feats_c = np.clip(feats, -FEAT_CLAMP, FEAT_CLAMP)
    spec = feats_c @ out_w + out_b                  # (T, D)
    out_flat += bk_scale[None, :] * spec

    return out_flat.reshape(B, N, D)


# revision 22
# speedup vs baseline: 1.0247x; 1.0024x over previous
"""MoE ResNet BK layer — Trainium2 Bass kernel (8 NeuronCores, expert-parallel).

Strategy:
  * Expert-parallel: core e holds expert e's (w1, b1, w2, b2) resident in SBUF.
  * Host computes the top-2 softmax gating and routes tokens: core e receives
    only the tokens whose top-2 set contains expert e (padded to capacity C),
    pre-transposed to [D, C] so the device keeps the hidden activations in
    [feature, token] layout throughout.
  * Device per core: h = gelu_tanh(w1e.T @ xg + b1e) in [F, tok] layout,
    y = w2e.T @ h + b2e in [D, tok] layout. Matmuls run as float32r
    (full-rate fp32, ~FP22 multiply precision, fp32 accumulate in PSUM).
  * Host combines: out[tok] = sum over the token's 2 experts of cw * y,
    plus the (cheap, serial) BK tridiagonal Green's-function branch.
"""

import numpy as np

B, N, D, E, F, K = 2, 2048, 768, 8, 3072, 2
V_MAX = 3.0
FEAT_CLAMP = 10.0
NCORES = 8
KD = D // 128   # 6
KF = F // 128   # 24

LAST_RESULTS = None  # BassKernelResults of the most recent device run


# ---------------------------------------------------------------- host pieces

def _softmax(x, axis=-1):
    m = np.max(x, axis=axis, keepdims=True)
    p = np.exp(x - m)
    return p / np.sum(p, axis=axis, keepdims=True)


def _route(x_flat, gate_w, gate_b):
    """Top-2 routing. Returns (top1, top2, cw1, cw2) per token."""
    logits = x_flat.astype(np.float32) @ gate_w.astype(np.float32) + gate_b
    probs = _softmax(logits, axis=-1)
    t = np.arange(probs.shape[0])
    top1 = np.argmax(probs, axis=-1)
    p2 = probs.copy()
    p2[t, top1] = -np.inf
    top2 = np.argmax(p2, axis=-1)
    v1 = probs[t, top1]
    v2 = probs[t, top2]
    s = v1 + v2
    return top1, top2, (v1 / s).astype(np.float32), (v2 / s).astype(np.float32)


def _bk_feats(x_flat, v_w, v_b):
    """BK tridiagonal Green's function diagonal -> (B*N, 2) float32 feats."""
    v = np.clip(x_flat @ v_w.astype(np.float64) + np.float64(v_b), -V_MAX, V_MAX)
    a = (-2.0 + v).astype(np.complex128) - 1j           # (B*N,)
    aT = a.reshape(B, N).T                              # (N, B)
    # Forward scan (d) and reverse scan (e) share one python loop.
    s = np.concatenate([aT, aT[::-1]], axis=1)          # (N, 2B)
    r = np.empty_like(s)
    r[0] = s[0]
    for i in range(1, N):
        r[i] = s[i] - 1.0 / r[i - 1]
    d = r[:, :B]
    e = r[::-1, B:]
    G = (1.0 / (d + e - aT)).T                          # (B, N)
    feats = np.stack([G.real, G.imag], axis=-1).astype(np.float32)
    return feats.reshape(B * N, 2)


def _round_fp22(a):
    """Round float32 -> fp32r (FP22: 13 mantissa bits), round-to-nearest-even."""
    b = np.ascontiguousarray(a, np.float32).view(np.uint32)
    keep = np.uint32(10)  # drop 10 low mantissa bits
    half = np.uint32(1 << 9)
    lsb = (b >> keep) & np.uint32(1)
    r = (b + (half - np.uint32(1)) + lsb) & np.uint32(~((1 << 10) - 1) & 0xFFFFFFFF)
    return r.view(np.float32)


# ---------------------------------------------------------------- device part

_BASS_CACHE = {}


def _build_bass(C, tiles):
    """Per-core expert-FFN program: yt = (gelu(xg@w1+b1) @ w2 + b2) in [D, C]."""
    import concourse.bacc as bacc
    import concourse.tile as tile
    import concourse.mybir as mybir

    f32 = mybir.dt.float32
    f32r = mybir.dt.float32r
    GELU = mybir.ActivationFunctionType.Gelu_apprx_tanh

    nc = bacc.Bacc("TRN2", target_bir_lowering=False, debug=False,
                   num_devices=NCORES)
    xt_d = nc.dram_tensor("xt", [D, C], f32r, kind="ExternalInput").ap()
    # w1 arrives pre-packed: row f*128+p holds w1[k*128+p, f*128+m] for all
    # (k, m), so each per-f tile DMA is one contiguous 384 KB burst.
    w1_d = nc.dram_tensor("w1e", [F, D], f32r, kind="ExternalInput").ap()
    w2_d = nc.dram_tensor("w2e", [F, D], f32r, kind="ExternalInput").ap()
    bc_d = nc.dram_tensor("bc", [128, KF + KD], f32,
                          kind="ExternalInput").ap()
    yt_d = nc.dram_tensor("yt", [D, C], f32, kind="ExternalOutput").ap()
    y_v = yt_d.rearrange("(d p) n -> p d n", p=128)          # [128, KD, C]

    # DRAM view tiled to 128-partition blocks.
    x_v = xt_d.rearrange("(k p) n -> p k n", p=128)          # [128, KD, C]

    with tile.TileContext(nc) as tc:
        with (
            tc.tile_pool(name="wts", bufs=1) as wpool,
            tc.tile_pool(name="xin", bufs=1) as xpool,
            tc.tile_pool(name="hbuf", bufs=3) as hpool,
            tc.tile_pool(name="yout", bufs=1) as ypool,
            tc.tile_pool(name="ph", bufs=2, space="PSUM") as phpool,
            tc.tile_pool(name="py", bufs=1, space="PSUM") as pypool,
        ):
            # PE warm-up: dummy fp32 matmuls keep the PE instruction stream
            # hot and flip the HAM clock-gate to 8/8 while the first
            # weight/x DMAs are still in flight.
            dmx = wpool.tile([128, 512], f32, tag="dmx")
            nc.gpsimd.memset(dmx[:], 0.0)
            dps = phpool.tile([128, 128], f32, tag="ph", name="dps")
            for i in range(12):
                nc.tensor.matmul(dps[:], dmx[:, 0:128], dmx[:, 128:256],
                                 start=(i == 0), stop=(i == 11))

            bc_sb = wpool.tile([128, KF + KD], f32, tag="bc")
            b1_sb = bc_sb[:, :KF]
            b2_sb = bc_sb[:, KF:]
            w1f = [wpool.tile([128, KD, 128], f32r, tag=f"w1_{f}",
                              name=f"w1_{f}") for f in range(KF)]
            w2f = [wpool.tile([128, D], f32r, tag=f"w2_{f}", name=f"w2_{f}")
                   for f in range(KF)]
            xts = [xpool.tile([128, KD, NT], f32r, tag=f"x{i}", name=f"x{i}")
                   for i, (t0, NT) in enumerate(tiles)]

            # All loads go through the single Sync HWDGE queue in explicit
            # priority order (one queue already sustains full HBM read BW;
            # parallel queues only starve the critical weight stream).
            def dma_x(i, k):
                t0, NT = tiles[i]
                if k is None:
                    nc.sync.dma_start(xts[i][:], x_v[:, :, t0:t0 + NT])
                else:
                    nc.sync.dma_start(xts[i][:, k, :],
                                      x_v[:, k, t0:t0 + NT])

            # x-tile i>0 is consumed ~60us per preceding tile; slot its
            # load into the weight stream well before it is needed.
            x_after = {}
            for i in range(1, len(tiles)):
                x_after.setdefault(min(10 * i, KF - 1), []).append(i)
            # The compute consumes weights staggered -- MM1(f+1) issues
            # before MM2(f) -- so deliver w1 one step ahead of w2.
            t0_, NT_ = tiles[0]
            nc.sync.dma_start(xts[0][:, :2, :], x_v[:, :2, t0_:t0_ + NT_])
            nc.sync.dma_start(w1f[0][:], w1_d[0:128, :])
            nc.sync.dma_start(bc_sb[:], bc_d[:])
            nc.sync.dma_start(xts[0][:, 2:, :], x_v[:, 2:, t0_:t0_ + NT_])
            for f in range(KF):
                if f + 1 < KF:
                    nc.sync.dma_start(w1f[f + 1][:],
                                      w1_d[(f + 1) * 128:(f + 2) * 128, :])
                nc.sync.dma_start(w2f[f][:], w2_d[f * 128:(f + 1) * 128, :])
                for i in x_after.get(f, ()):
                    dma_x(i, None)

            for i, (t0, NT) in enumerate(tiles):
                xt_sb = xts[i]
                pys = [pypool.tile([128, NT], f32, tag=f"py{d}", name=f"py{d}")
                       for d in range(KD)]
                pending = None  # (h_tile, f) whose MM2 is not yet emitted
                for f in range(KF):
                    ph = phpool.tile([128, NT], f32, tag="ph")
                    for k in range(KD):
                        nc.tensor.matmul(
                            ph[:],
                            w1f[f][:, k, :],
                            xt_sb[:, k, :],
                            start=(k == 0), stop=(k == KD - 1),
                        )
                    hf = hpool.tile([128, NT], f32r, tag="h")
                    nc.scalar.activation(hf[:], ph[:], GELU,
                                         bias=b1_sb[:, f:f + 1])
                    if pending is not None:
                        hp, fp = pending
                        for d in range(KD):
                            nc.tensor.matmul(
                                pys[d][:],
                                w2f[fp][:, d * 128:(d + 1) * 128],
                                hp[:],
                                start=(fp == 0), stop=False,
                            )
                    pending = (hf, f)
                hp, fp = pending
                for d in range(KD):
                    nc.tensor.matmul(
                        pys[d][:],
                        w2f[fp][:, d * 128:(d + 1) * 128],
                        hp[:],
                        start=False, stop=True,
                    )
                IDENT = mybir.ActivationFunctionType.Identity
                y_all = ypool.tile([128, KD, NT], f32, tag="y")
                for d in (0, 1, 2, 3, 4, 5):
                    # Split the PSUM-draining b2-add across DVE and ACT so
                    # the psum banks free up ~2x faster for the next tile.
                    if d % 2:
                        nc.scalar.activation(y_all[:, d, :], pys[d][:], IDENT,
                                             bias=b2_sb[:, d:d + 1])
                    else:
                        nc.vector.tensor_scalar_add(y_all[:, d, :], pys[d][:],
                                                    b2_sb[:, d:d + 1])
                half = KD // 2
                nc.sync.dma_start(y_v[:, :half, t0:t0 + NT],
                                  y_all[:, :half, :])
                nc.sync.dma_start(y_v[:, half:, t0:t0 + NT],
                                  y_all[:, half:, :])
    nc.compile()
    return nc


def _token_tiles(C):
    """Decompose C into moving-dim tiles, all >=256 (fp32r full rate)."""
    tiles = []
    t0 = 0
    rem = C
    while rem >= 768:
        tiles.append((t0, 512))
        t0 += 512
        rem -= 512
    if rem > 512:
        a = rem - 256
        tiles += [(t0, a), (t0 + a, 256)]
    elif rem > 0:
        tiles.append((t0, rem))
    return tiles


# ---------------------------------------------------------------- entry point

def kernel(x, v_w, v_b, gate_w, gate_b, w1, b1, w2, b2, out_w, out_b, bk_scale):
    global LAST_RESULTS
    from concourse.bass_utils import run_bass_kernel_spmd

    x = np.asarray(x, np.float32)
    v_w = np.asarray(v_w, np.float32)
    v_b = np.float32(np.asarray(v_b))
    gate_w = np.asarray(gate_w, np.float32)
    gate_b = np.asarray(gate_b, np.float32)
    w1 = np.asarray(w1, np.float32)
    b1 = np.asarray(b1, np.float32)
    w2 = np.asarray(w2, np.float32)
    b2 = np.asarray(b2, np.float32)
    out_w = np.asarray(out_w, np.float32)
    out_b = np.asarray(out_b, np.float32)
    bk_scale = np.asarray(bk_scale, np.float32)

    T = B * N
    x_flat = np.ascontiguousarray(x.reshape(T, D))

    # --- host: gating / routing ---
    top1, top2, cw1, cw2 = _route(x_flat, gate_w, gate_b)
    idx = []
    cws = []
    for e in range(E):
        m1 = top1 == e
        m2 = top2 == e
        ie = np.nonzero(m1 | m2)[0]
        ce = np.where(m1[ie], cw1[ie], cw2[ie]).astype(np.float32)
        idx.append(ie)
        cws.append(ce)
    max_load = max(len(ie) for ie in idx)
    C = max(256, -(-max_load // 8) * 8)
    tiles = _token_tiles(C)

    # --- device: expert FFN on 8 cores ---
    key = (C,)
    nc = _BASS_CACHE.get(key)
    if nc is None:
        nc = _build_bass(C, tiles)
        _BASS_CACHE[key] = nc

    in_maps = []
    for e in range(E):
        ie = idx[e]
        xt = np.zeros((D, C), np.float32)
        xt[:, :len(ie)] = _round_fp22(x_flat[ie].T)
        in_maps.append({
            "xt": xt,
            "w1e": _round_fp22(w1[e].reshape(KD, 128, KF, 128)
                               .transpose(2, 1, 0, 3).reshape(F, D)),
            "w2e": _round_fp22(w2[e]),
            "bc": np.ascontiguousarray(np.concatenate(
                [b1[e].reshape(KF, 128).T, b2[e].reshape(KD, 128).T],
                axis=1)),
        })

    res = run_bass_kernel_spmd(nc, in_maps, core_ids=list(range(NCORES)))
    LAST_RESULTS = res

    # --- host: combine expert outputs ---
    out_flat = np.zeros((T, D), np.float32)
    for e in range(E):
        ie = idx[e]
        ye = res.results[e]["yt"][:, :len(ie)].T   # (n_e, D)
        out_flat[ie] += cws[e][:, None] * ye

    # --- host: BK spectral branch ---
    feats = _bk_feats(x_flat.astype(np.float64), v_w, v_b)
    feats_c = np.clip(feats, -FEAT_CLAMP, FEAT_CLAMP)
    spec = feats_c @ out_w + out_b                  # (T, D)
    out_flat += bk_scale[None, :] * spec

    return out_flat.reshape(B, N, D)
